# revision 23
# baseline (speedup 1.0000x reference)
"""Trainium2 Bass kernel for multi-head attention with RoPE.

Problem: B=2, T=2048, D=1024, H=16 heads (hd=64), fused qkv projection,
RoPE on q/k, softmax attention, output projection.

Sharding over 8 NeuronCores: data parallel on batch (2) x tensor parallel
on heads (4 groups of 4 heads). Core c handles batch c//4, head group c%4:
 - qkv projection: w_qkv column-split per head group
 - attention for its 4 heads
 - out projection: w_out row-split; per-core partial [T, D] outputs are
   summed on the host (the TP all-reduce is folded into the host gather).

v2 redesign (from perfetto analysis of the 255us v1):
 - PE warm-up matmuls at t=0: the HAM clock gate keeps PE at 1.2 GHz until
   ~3.4us of sustained activity; v1 ran its whole prologue cold.
 - inputs host-packed into ~12 large DMAs (v1: ~36 small ones at ~600ns
   sync-engine issue cost each, serializing the prologue).
 - ONE continuous attention stream across all 8 (head, ih) units with the
   PV matmuls lagging LAG j-blocks behind scores/exp, crossing unit
   boundaries (v1 drained each unit separately, diluting PE density and
   letting the clock gate go cold at unit seams).
 - qk bias folded into the RoPE scalar_tensor_tensor ops (v1: extra bias
   matmuls); v bias dropped on device entirely: softmax rows sum to 1, so
   bias_v contributes bv^T @ w_out, a constant row added on the host.
 - softmax normalize: DVE reciprocal -> PE outer-product broadcast of the
   denominator row -> DVE multiply (v1 used a DMA hop + gpsimd broadcast,
   an ~8us serial chain at the tail that also let the PE clock go cold).
 - cos/sin tables shipped bf16 (halves their DMA bytes).

Layout per core (same as v1):
 - x pre-transposed on host: xT [D, T]; q,k computed transposed [c, t]
   (c on partitions) so RoPE's rotate-half is a partition swap (SBUF-SBUF
   DMA) and scores need no transposes.
 - scores zero-pad k to K=128 (K=64 matmuls take a slow weight path).
 - softmax denominator from an all-ones column appended to v; PV yields
   [65, T]: rows 0..63 unnormalized out^T, row 64 the denominator.
 - no max-subtraction: logits ~N(0,1), exp safe in f32.
 - bf16 matmul inputs, f32 PSUM accumulation.
"""

import sys
import numpy as np

for _p in ("/opt/trn_rl_repo", "/root/.axon_site/_ro/trn_rl_repo"):
    if _p not in sys.path:
        sys.path.insert(0, _p)

import ml_dtypes

BF16 = ml_dtypes.bfloat16

B, T, D, H = 2, 2048, 1024, 16
HD = 64          # head dim
HG = 4           # heads per core (group)
CQK = 512        # q cols + k cols per core
CV = 256         # v cols per core
N_CORES = 8
KT = 8           # number of 128-row d-chunks (D / 128)
IH = 1024        # ih-half width of the attention stream
NJ = T // 128    # j-blocks per unit
MAXLAG = 16      # max PV lag behind scores/exp, in j-blocks
NWARM = 14       # HAM warm-up matmuls


class _Builder:
    def build(self):
        import concourse.mybir as mybir
        from concourse import bacc
        from concourse.tile import TileContext

        f32 = mybir.dt.float32
        bf16 = mybir.dt.bfloat16
        EXP = mybir.ActivationFunctionType.Exp
        ADD = mybir.AluOpType.add
        MULT = mybir.AluOpType.mult

        nc = bacc.Bacc()

        wqk_e = nc.declare_dram_parameter("wqk", [128, KT * CQK], bf16, isOutput=False)
        xh0_e = nc.declare_dram_parameter("xh0", [128, KT * IH], bf16, isOutput=False)
        xh1_e = nc.declare_dram_parameter("xh1", [128, KT * IH], bf16, isOutput=False)
        csA_e = nc.declare_dram_parameter("csA", [128, 2 * IH], bf16, isOutput=False)
        csB_e = nc.declare_dram_parameter("csB", [128, 2 * IH], bf16, isOutput=False)
        bqk_e = nc.declare_dram_parameter("bqk", [128, 4], f32, isOutput=False)
        wv_e = nc.declare_dram_parameter("wv", [128, KT * CV], bf16, isOutput=False)
        wo_e = nc.declare_dram_parameter("wo", [128, 2 * D], bf16, isOutput=False)
        y_e = nc.declare_dram_parameter("y", [T, D], bf16, isOutput=True)

        tc = TileContext(nc)
        tc.__enter__()

        pool_live = tc.alloc_tile_pool(name="live", bufs=1)
        pool_a = tc.alloc_tile_pool(name="stageA", bufs=1)
        pool_ps = tc.alloc_tile_pool(name="psum", bufs=1, space="PSUM")

        # ---------------- warm-up: flip the HAM clock gate early -----------
        wtile = pool_a.tile([128, 512], bf16, name="wtile", tag="wtile")
        nc.vector.memset(wtile[:, :], 0.0078125)
        for _ in range(NWARM):
            ps_w = pool_ps.tile([128, 256], f32, name="psw", tag="slot2", bufs=3)
            nc.tensor.matmul(ps_w[:, :], wtile[:, 0:128], wtile[:, 0:256],
                             start=True, stop=True)

        # ---------------- input loads (few, large DMAs) --------------------
        bqk_t = pool_a.tile([128, 4], f32, name="bqk_t", tag="bqk_t")
        nc.sync.dma_start(out=bqk_t[:, :], in_=bqk_e[:, :])
        # wqk packed k-block (cols 0:2048) then q-block; k loads first since
        # the first kpad needs only k
        wqk_t = pool_a.tile([128, KT * CQK], bf16, name="wqk_t", tag="wqk_t")
        nc.sync.dma_start(out=wqk_t[:, 0:2048], in_=wqk_e[:, 0:2048])
        xh = []
        for hb, xe in ((0, xh0_e), (1, xh1_e)):
            xt = pool_a.tile([128, KT * IH], bf16, name=f"xh{hb}", tag=f"xh{hb}")
            xh.append(xt)
        # prologue-critical loads only; the rest are issued after the
        # prologue so its rope-swap DMAs aren't queued behind bulk input
        cs_t = []
        csA_t = pool_a.tile([128, 2 * IH], bf16, name="csA", tag="csA")
        nc.sync.dma_start(out=xh[0][:, 0:4 * IH], in_=xh0_e[:, 0:4 * IH])
        nc.sync.dma_start(out=csA_t[:, :], in_=csA_e[:, :])
        nc.sync.dma_start(out=xh[0][:, 4 * IH:], in_=xh0_e[:, 4 * IH:])
        nc.sync.dma_start(out=wqk_t[:, 2048:], in_=wqk_e[:, 2048:])
        cs_t.append(csA_t)
        csB_t = pool_a.tile([128, 2 * IH], bf16, name="csB", tag="csB")
        cs_t.append(csB_t)
        wv_t = pool_a.tile([128, KT * CV], bf16, name="wv_t", tag="wv_t")
        wo_t = pool_live.tile([128, 2 * D], bf16, name="wo_t", tag="wo_t")

        # row 64 of all-ones: matmul lhsT/rhs must share a base partition,
        # and the denominator-reciprocal row lives on partition 64
        ones64 = pool_live.tile([65, 64], f32, name="ones64", tag="ones64")
        nc.vector.memset(ones64[64:65, :], 1.0)

        # ---------------- persistent SBUF tiles ----------------------------
        qk_rot = []
        for cb in range(4):
            qr = pool_live.tile([128, T], bf16, name=f"qkrot{cb}", tag=f"qkrot{cb}")
            qk_rot.append(qr)
        v_sb = []
        for tb in range(NJ):
            vt = pool_live.tile([128, HG, 65], bf16, name=f"v{tb}", tag=f"v{tb}")
            v_sb.append(vt)
        attn = []
        for pair in range(2):
            at = pool_live.tile([128, T], bf16, name=f"attn{pair}", tag=f"attn{pair}")
            attn.append(at)

        # ---------------- building blocks ----------------------------------
        def a_chunk_mm(cb, hb, dlo, dhi, ps_box):
            """q/k projection matmuls for d-chunks [dlo, dhi)."""
            if ps_box[0] is None:
                ps_box[0] = pool_ps.tile([128, IH], f32, name="ps2",
                                         tag="slot2", bufs=3)
            ps_qk = ps_box[0]
            for di in range(dlo, dhi):
                c0 = (di * 256 + (cb - 2) * 128) if cb >= 2 \
                    else (2048 + di * 256 + cb * 128)
                for half in range(2):
                    nc.tensor.matmul(
                        ps_qk[:, half * 512:(half + 1) * 512],
                        wqk_t[:, c0:c0 + 128],
                        xh[hb][:, di * IH + half * 512: di * IH + (half + 1) * 512],
                        start=(di == 0), stop=(di == KT - 1),
                    )

        def a_chunk_rope(cb, hb, ps_box):
            """RoPE + bias on the finished projection chunk."""
            ps_qk = ps_box[0]
            cst = cs_t[hb]
            bias = bqk_t[:, cb:cb + 1]
            # rot[p] = (q[p]+b[p])*cos[p] + swap((q+b)*sin_signed)[p]
            tmp1 = pool_a.tile([128, IH], f32, name="ropet1", tag="ropet1", bufs=2)
            tmps = pool_a.tile([128, IH], f32, name="tmps", tag="tmps", bufs=2)
            nc.vector.scalar_tensor_tensor(tmp1[:, :], ps_qk[:, :], bias,
                                           cst[:, 0:IH], op0=ADD, op1=MULT)
            nc.vector.scalar_tensor_tensor(tmps[:, :], ps_qk[:, :], bias,
                                           cst[:, IH:2 * IH], op0=ADD, op1=MULT)
            qsw = pool_a.tile([128, IH], f32, name="qsw", tag="qsw", bufs=2)
            for blk in range(2):
                b0 = blk * 64
                nc.sync.dma_start(out=qsw[b0:b0 + 32, :], in_=tmps[b0 + 32:b0 + 64, :])
                nc.sync.dma_start(out=qsw[b0 + 32:b0 + 64, :], in_=tmps[b0:b0 + 32, :])
            nc.vector.tensor_add(qk_rot[cb][:, hb * IH:(hb + 1) * IH],
                                 tmp1[:, :], qsw[:, :])
            ps_box[0] = None

        def a_chunk(cb, hb):
            box = [None]
            a_chunk_mm(cb, hb, 0, KT, box)
            a_chunk_rope(cb, hb, box)

        def a_parts(cb, hb):
            """a_chunk split into 3 filler closures."""
            box = [None]
            return [lambda: a_chunk_mm(cb, hb, 0, 4, box),
                    lambda: a_chunk_mm(cb, hb, 4, KT, box),
                    lambda: a_chunk_rope(cb, hb, box)]

        def b_unit(tb):
            """v projection for one t-block (no bias: folded to host)."""
            hb, c0 = tb // 8, (tb % 8) * 128
            ps_v = pool_ps.tile([128, CV], f32, name="psv", tag="slot2", bufs=3)
            for di in range(KT):
                nc.tensor.matmul(
                    ps_v[:, :],
                    xh[hb][:, di * IH + c0: di * IH + c0 + 128],
                    wv_t[:, di * CV:(di + 1) * CV],
                    start=(di == 0), stop=(di == KT - 1),
                )
            nc.vector.tensor_copy(
                v_sb[tb][:, :, 0:64],
                ps_v.rearrange("p (h d) -> p h d", h=HG),
            )
            nc.vector.memset(v_sb[tb][:, :, 64:65], 1.0)

        def d_unit(tb, copy_eng="v"):
            """out projection for one t-block; partial y to DRAM in bf16."""
            ps_y = pool_ps.tile([128, D], f32, name="psy", tag="slot2", bufs=3)
            for kb in range(2):
                for nb in range(D // 512):
                    nsl = slice(nb * 512, (nb + 1) * 512)
                    nc.tensor.matmul(
                        ps_y[:, nsl],
                        attn[kb][:, tb * 128:(tb + 1) * 128],
                        wo_t[:, kb * D + nb * 512: kb * D + (nb + 1) * 512],
                        start=(kb == 0), stop=(kb == 1),
                    )
            y_sb = pool_live.tile([128, D], bf16, name="y_sb", tag="y_sb", bufs=3)
            with nc.allow_low_precision("bf16 partial output; host sums in f32"):
                if copy_eng == "v":
                    nc.vector.tensor_copy(y_sb[:, :], ps_y[:, :])
                else:
                    nc.scalar.copy(y_sb[:, :], ps_y[:, :])
            nc.sync.dma_start(out=y_e[tb * 128:(tb + 1) * 128, :], in_=y_sb[:, :])

        kpads = {}

        def kpad_half(h, half):
            """Zero-padded k for head h, built one T-half at a time.

            One buffer per head (bufs=4), built once, reused by both ih
            units — rebuilds cost DVE time and stall the PE FIFO.
            """
            pair, hh = h // 2, h % 2
            hp = hh * 64
            if half == 0:
                kpads[h] = pool_live.tile([128, T], bf16, name="kpad",
                                          tag="kpad", bufs=4)
            kp = kpads[h]
            sl = slice(half * IH, (half + 1) * IH)
            op = 64 - hp
            nc.vector.memset(kp[op:op + 64, sl], 0.0)
            nc.vector.tensor_copy(kp[hp:hp + 64, sl],
                                  qk_rot[2 + pair][hp:hp + 64, sl])

        # stream state
        pt_tiles = {}
        ps_o_box = [None]

        def scores_exp(u, j):
            h, ih = u
            pair = h // 2
            ps_s = pool_ps.tile([128, IH], f32, name="pss", tag="slot2", bufs=3)
            pt = pool_live.tile([128, IH], bf16, name="pt", tag="pt",
                                bufs=MAXLAG + 4)
            kp = kpads[h]
            jsl = slice(j * 128, (j + 1) * 128)
            for nb in range(2):
                nsl = slice(nb * 512, (nb + 1) * 512)
                nc.tensor.matmul(ps_s[:, nsl], kp[:, jsl],
                                 qk_rot[pair][:, ih * IH + nb * 512:
                                              ih * IH + (nb + 1) * 512],
                                 start=True, stop=True)
            nc.scalar.activation(pt[:, :], ps_s[:, :], EXP, scale=0.125)
            pt_tiles[(u, j)] = pt

        def pv(u, j):
            h, ih = u
            if ps_o_box[0] is None:
                ps_o_box[0] = pool_ps.tile([65, IH], f32, name="pso",
                                           tag="ps_o", bufs=1)
            ps_o = ps_o_box[0]
            pt = pt_tiles.pop((u, j))
            for nb in range(2):
                nsl = slice(nb * 512, (nb + 1) * 512)
                nc.tensor.matmul(ps_o[:, nsl], v_sb[j][:, h, :], pt[:, nsl],
                                 start=(j == 0), stop=(j == NJ - 1))

        def normalize(u):
            h, ih = u
            pair, hh = h // 2, h % 2
            isl = slice(ih * IH, (ih + 1) * IH)
            ps_o = ps_o_box[0]
            # drain ps_o to SBUF first: frees the PV accumulator after one
            # copy (the next unit's PV j=0 reuses it almost immediately),
            # and gives the final multiply its one-PSUM-operand legality
            od = pool_live.tile([65, IH], f32, name="odrain", tag="odrain",
                                bufs=2)
            nc.vector.tensor_copy(od[:, :], ps_o[:, :])
            ps_o_box[0] = None
            rt = pool_live.tile([65, IH], f32, name="recip_t", tag="recip_t",
                                bufs=2)
            with nc.allow_low_precision("bf16 softmax normalization"):
                nc.vector.reciprocal_approx_fast(out=rt[:, :], in_=od[:, :])
            # broadcast the denominator-reciprocal row across 64 partitions
            # with a PE outer product (v1 used a DMA hop + gpsimd broadcast)
            ps_b = pool_ps.tile([64, IH], f32, name="psb", tag="slot2", bufs=3)
            for nb in range(2):
                nsl = slice(nb * 512, (nb + 1) * 512)
                nc.tensor.matmul(ps_b[:, nsl], ones64[64:65, :], rt[64:65, nsl],
                                 start=True, stop=True)
            with nc.allow_low_precision("bf16 attention output"):
                if hh == 0:
                    nc.vector.tensor_mul(attn[pair][0:64, isl],
                                         od[0:64, :], ps_b[:, :])
                else:
                    atmp = pool_live.tile([64, IH], bf16, name="atmp",
                                          tag="atmp", bufs=2)
                    nc.vector.tensor_mul(atmp[:, :], od[0:64, :], ps_b[:, :])
                    nc.sync.dma_start(out=attn[pair][64:128, isl], in_=atmp[:, :])

        # ---------------- prologue ------------------------------------------
        a_chunk(2, 0)        # k pair0, T-half 0
        kpad_half(0, 0)      # before a(0,0)'s rope in the DVE FIFO
        a_chunk(0, 0)        # q pair0, T-half 0
        # bulk loads not needed by the prologue, issued after its rope DMAs
        nc.sync.dma_start(out=xh[1][:, 0:4 * IH], in_=xh1_e[:, 0:4 * IH])
        nc.sync.dma_start(out=xh[1][:, 4 * IH:], in_=xh1_e[:, 4 * IH:])
        nc.sync.dma_start(out=wv_t[:, :], in_=wv_e[:, :])
        nc.sync.dma_start(out=csB_t[:, :], in_=csB_e[:, :])
        nc.sync.dma_start(out=wo_t[:, :], in_=wo_e[:, :])

        # ---------------- stream fillers, with PE-cost annotations ----------
        from collections import defaultdict
        fill = defaultdict(list)
        fcost = defaultdict(int)

        def put(p, c, cost=0):
            fill[p].append(c)
            fcost[p] += cost

        APART_NS = 1728   # 8 projection matmuls of N=512
        B_NS = 900        # b_unit: 8 matmuls of N=256 + copy
        D_NS = 950        # d_unit: 4 matmuls of N=512 + copy + dma

        pa = a_parts(2, 1)
        put(0, pa[0], APART_NS); put(2, pa[1], APART_NS); put(4, pa[2])
        put(3, lambda: kpad_half(1, 0))
        put(5, lambda: kpad_half(0, 1))
        put(6, lambda: kpad_half(1, 1))
        pa = a_parts(3, 0)
        put(7, pa[0], APART_NS); put(9, pa[1], APART_NS); put(11, pa[2])
        pa = a_parts(3, 1)
        put(12, pa[0], APART_NS); put(14, pa[1], APART_NS); put(16, pa[2])
        pa = a_parts(1, 0)
        put(18, pa[0], APART_NS); put(20, pa[1], APART_NS); put(22, pa[2])
        put(24, lambda: kpad_half(2, 0))
        put(26, lambda: kpad_half(2, 1))
        put(28, lambda: kpad_half(3, 0))
        put(30, lambda: kpad_half(3, 1))
        pa = a_parts(0, 1)
        put(50, pa[0], APART_NS); put(52, pa[1], APART_NS); put(54, pa[2])
        pa = a_parts(1, 1)
        put(68, pa[0], APART_NS); put(70, pa[1], APART_NS); put(72, pa[2])
        for tb in range(NJ):
            put(tb, lambda tb=tb: b_unit(tb), B_NS)
        # out-projection for t-blocks 0..5; idx 63's normalize is forced by
        # MAXLAG to land by p75, so p80+ can't head-of-line-block the PE
        for i, tb in enumerate(range(6)):
            put(80 + 5 * i, lambda tb=tb: d_unit(tb, "v"), D_NS)

        # ---------------- PV leveling plan ----------------------------------
        # defer PV matmuls out of overloaded early periods and catch up in
        # the slack, so per-period PE work tracks the ~1150ns exp cadence
        SCORES_NS = 432
        PV_NS = 432
        CAP = 1320
        NP = 128
        pv_plan = defaultdict(list)
        ptr = 0
        p = 0
        while ptr < NP:
            pe = (SCORES_NS if p < NP else 0) + fcost[p]
            while ptr < NP and ptr + 2 <= p and len(pv_plan[p]) < 3:
                if (p - ptr) >= MAXLAG or pe + PV_NS <= CAP or p >= NP:
                    pv_plan[p].append(ptr)
                    pe += PV_NS
                    ptr += 1
                else:
                    break
            p += 1
        NPTOT = p

        # ---------------- the stream ----------------------------------------
        units = [(0, 0), (1, 0), (2, 0), (3, 0), (0, 1), (1, 1), (2, 1), (3, 1)]
        for p in range(NPTOT):
            if p < NP:
                scores_exp(units[p // NJ], p % NJ)
            for f in fill[p]:
                f()
            for idx in pv_plan[p]:
                uq = units[idx // NJ]
                pv(uq, idx % NJ)
                if idx % NJ == NJ - 1:
                    normalize(uq)

        # ---------------- tail ----------------------------------------------
        pool_a.release()
        for i, tb in enumerate(range(6, 16)):
            d_unit(tb, "v" if i % 2 == 0 else "s")

        pool_ps.release()
        pool_live.release()
        tc.__exit__(None, None, None)
        nc.finalize()
        return nc


def make_inputs(x, w_qkv, b_qkv, w_out):
    """Host-side shard prep. Returns in_maps list for the 8 cores."""
    half = HD // 2
    inv = 1.0 / (10000.0 ** (np.arange(half, dtype=np.float32) / half))
    fr = np.arange(T, dtype=np.float32)[:, None] * inv[None, :]   # [T, 32]
    cosT = np.cos(fr).T                                           # [32, T]
    sinT = np.sin(fr).T
    cos128 = np.tile(cosT, (4, 1)).astype(np.float32)             # [128, T]
    sin128 = np.tile(sinT, (4, 1)).astype(np.float32)
    sign = np.where((np.arange(128) % 64) < 32, 1.0, -1.0).astype(np.float32)
    sin128 = sin128 * sign[:, None]

    in_maps = []
    for c in range(N_CORES):
        b, g = c // 4, c % 4
        qcols = slice(g * 256, (g + 1) * 256)
        kcols = slice(D + g * 256, D + (g + 1) * 256)
        vcols = slice(2 * D + g * 256, 2 * D + (g + 1) * 256)

        wqk = np.concatenate([w_qkv[:, qcols], w_qkv[:, kcols]], axis=1)
        bqk = np.concatenate([b_qkv[qcols], b_qkv[kcols]])        # [512]
        wv = w_qkv[:, vcols]                                      # [D, 256]
        wo = w_out[g * 256:(g + 1) * 256, :]                      # [256, D]
        xT = np.ascontiguousarray(x[b].T).astype(BF16)            # [D, T]

        wqk_pack = np.empty((128, KT * CQK), dtype=BF16)
        wv_pack = np.empty((128, KT * CV), dtype=BF16)
        xh0 = np.empty((128, KT * IH), dtype=BF16)
        xh1 = np.empty((128, KT * IH), dtype=BF16)
        for di in range(KT):
            rs = slice(di * 128, (di + 1) * 128)
            # k-block (chunk cols 256:512) first, q-block second — the
            # device loads k first for the first kpad
            wqk_pack[:, di * 256:(di + 1) * 256] = wqk[rs, 256:512].astype(BF16)
            wqk_pack[:, 2048 + di * 256: 2048 + (di + 1) * 256] = \
                wqk[rs, 0:256].astype(BF16)
            wv_pack[:, di * CV:(di + 1) * CV] = wv[rs, :].astype(BF16)
            xh0[:, di * IH:(di + 1) * IH] = xT[rs, 0:IH]
            xh1[:, di * IH:(di + 1) * IH] = xT[rs, IH:T]
        bqkT = np.empty((128, 4), dtype=np.float32)
        for cb in range(4):
            bqkT[:, cb] = bqk[cb * 128:(cb + 1) * 128]
        csA = np.concatenate([cos128[:, 0:IH], sin128[:, 0:IH]],
                             axis=1).astype(BF16)
        csB = np.concatenate([cos128[:, IH:T], sin128[:, IH:T]],
                             axis=1).astype(BF16)
        wo_pack = np.empty((128, 2 * D), dtype=BF16)
        for kb in range(2):
            wo_pack[:, kb * D:(kb + 1) * D] = wo[kb * 128:(kb + 1) * 128, :]

        in_maps.append({
            "wqk": wqk_pack, "xh0": xh0, "xh1": xh1,
            "csA": csA, "csB": csB, "bqk": bqkT,
            "wv": wv_pack, "wo": wo_pack,
        })
    return in_maps


_NC_CACHE = [None]


def get_graph():
    if _NC_CACHE[0] is None:
        _NC_CACHE[0] = _Builder().build()
    return _NC_CACHE[0]


def kernel(x, w_qkv, b_qkv, w_out, b_out, _trace=False):
    from concourse.bass_utils import run_bass_kernel_spmd

    x = np.asarray(x)
    w_qkv = np.asarray(w_qkv)
    b_qkv = np.asarray(b_qkv)
    w_out = np.asarray(w_out)
    b_out = np.asarray(b_out)

    nc = get_graph()
    in_maps = make_inputs(x, w_qkv, b_qkv, w_out)
    kw = {}
    if _trace:
        _install_ntff_shim()
        kw = {"trace": True}
    res = run_bass_kernel_spmd(nc, in_maps, core_ids=list(range(N_CORES)), **kw)

    # v bias folded here: softmax rows sum to 1, so bias_v contributes
    # exactly bv^T @ w_out to every token's output.
    bias_row = (b_qkv[2 * D:].astype(np.float64) @
                w_out.astype(np.float64)).astype(np.float32) + b_out
    out = np.empty((B, T, D), dtype=np.float32)
    for b in range(B):
        acc = np.asarray(res.results[4 * b]["y"]).astype(np.float32)
        for g in range(1, 4):
            acc += np.asarray(res.results[4 * b + g]["y"]).astype(np.float32)
        out[b] = acc + bias_row[None, :]
    if _trace:
        kernel.last_exec_time_ns = res.exec_time_ns
        kernel.last_result = res
    return out


def _install_ntff_shim():
    """The agent image's antenv lacks axon_hooks; shim it so trace=True works."""
    import types
    if "antenv.axon_hooks" in sys.modules:
        return
    try:
        from trn_agent_boot.trn_boot import _ntff_profile_via_ctypes
        hook = _ntff_profile_via_ctypes("/opt/axon/libaxon_pjrt.so")
    except Exception:
        hook = None
    mod = types.ModuleType("antenv.axon_hooks")
    _h = [hook]
    mod.set_axon_ntff_profile_hook = lambda h: _h.__setitem__(0, h)
    mod.get_axon_ntff_profile_hook = lambda: _h[0]
    sys.modules["antenv.axon_hooks"] = mod


# revision 24
# speedup vs baseline: 1.3151x; 1.3151x over previous
"""Trainium2 Bass kernel for multi-head attention with RoPE.

Problem: B=2, T=2048, D=1024, H=16 heads (hd=64), fused qkv projection,
RoPE on q/k, softmax attention, output projection.

Sharding over 8 NeuronCores: data parallel on batch (2) x tensor parallel
on heads (4 groups of 4 heads). Core c handles batch c//4, head group c%4:
 - qkv projection: w_qkv column-split per head group
 - attention for its 4 heads
 - out projection: w_out row-split; per-core partial [T, D] outputs are
   summed on the host (the TP all-reduce is folded into the host gather).

v2 redesign (from perfetto analysis of the 255us v1):
 - PE warm-up matmuls at t=0: the HAM clock gate keeps PE at 1.2 GHz until
   ~3.4us of sustained activity; v1 ran its whole prologue cold.
 - inputs host-packed into ~12 large DMAs (v1: ~36 small ones at ~600ns
   sync-engine issue cost each, serializing the prologue).
 - ONE continuous attention stream across all 8 (head, ih) units with the
   PV matmuls lagging LAG j-blocks behind scores/exp, crossing unit
   boundaries (v1 drained each unit separately, diluting PE density and
   letting the clock gate go cold at unit seams).
 - qk bias folded into the RoPE scalar_tensor_tensor ops (v1: extra bias
   matmuls); v bias dropped on device entirely: softmax rows sum to 1, so
   bias_v contributes bv^T @ w_out, a constant row added on the host.
 - softmax normalize: DVE reciprocal -> PE outer-product broadcast of the
   denominator row -> DVE multiply (v1 used a DMA hop + gpsimd broadcast,
   an ~8us serial chain at the tail that also let the PE clock go cold).
 - cos/sin tables shipped bf16 (halves their DMA bytes).

Layout per core (same as v1):
 - x pre-transposed on host: xT [D, T]; q,k computed transposed [c, t]
   (c on partitions) so RoPE's rotate-half is a partition swap (SBUF-SBUF
   DMA) and scores need no transposes.
 - scores zero-pad k to K=128 (K=64 matmuls take a slow weight path).
 - softmax denominator from an all-ones column appended to v; PV yields
   [65, T]: rows 0..63 unnormalized out^T, row 64 the denominator.
 - no max-subtraction: logits ~N(0,1), exp safe in f32.
 - bf16 matmul inputs, f32 PSUM accumulation.
"""

import sys
import numpy as np

for _p in ("/opt/trn_rl_repo", "/root/.axon_site/_ro/trn_rl_repo"):
    if _p not in sys.path:
        sys.path.insert(0, _p)

import ml_dtypes

BF16 = ml_dtypes.bfloat16

B, T, D, H = 2, 2048, 1024, 16
HD = 64          # head dim
HG = 4           # heads per core (group)
CQK = 512        # q cols + k cols per core
CV = 256         # v cols per core
N_CORES = 8
KT = 8           # number of 128-row d-chunks (D / 128)
IH = 1024        # ih-half width of the attention stream
NJ = T // 128    # j-blocks per unit
MAXLAG = 16      # max PV lag behind scores/exp, in j-blocks
NWARM = 14       # HAM warm-up matmuls


class _Builder:
    def build(self):
        import concourse.mybir as mybir
        from concourse import bacc
        from concourse.tile import TileContext

        f32 = mybir.dt.float32
        bf16 = mybir.dt.bfloat16
        EXP = mybir.ActivationFunctionType.Exp
        ADD = mybir.AluOpType.add
        MULT = mybir.AluOpType.mult

        nc = bacc.Bacc()

        wqk_e = nc.declare_dram_parameter("wqk", [128, KT * CQK], bf16, isOutput=False)
        xh0_e = nc.declare_dram_parameter("xh0", [128, KT * IH], bf16, isOutput=False)
        xh1_e = nc.declare_dram_parameter("xh1", [128, KT * IH], bf16, isOutput=False)
        csA_e = nc.declare_dram_parameter("csA", [128, 2 * IH], bf16, isOutput=False)
        csB_e = nc.declare_dram_parameter("csB", [128, 2 * IH], bf16, isOutput=False)
        bqk_e = nc.declare_dram_parameter("bqk", [128, 4], f32, isOutput=False)
        wv_e = nc.declare_dram_parameter("wv", [128, KT * CV], bf16, isOutput=False)
        wo_e = nc.declare_dram_parameter("wo", [128, 2 * D], bf16, isOutput=False)
        y_e = nc.declare_dram_parameter("y", [T, D], bf16, isOutput=True)

        tc = TileContext(nc)
        tc.__enter__()

        pool_live = tc.alloc_tile_pool(name="live", bufs=1)
        pool_a = tc.alloc_tile_pool(name="stageA", bufs=1)
        pool_ps = tc.alloc_tile_pool(name="psum", bufs=1, space="PSUM")

        # ---------------- warm-up: flip the HAM clock gate early -----------
        wtile = pool_a.tile([128, 512], bf16, name="wtile", tag="wtile")
        nc.vector.memset(wtile[:, :], 0.0078125)
        for _ in range(NWARM):
            ps_w = pool_ps.tile([128, 256], f32, name="psw", tag="slot2", bufs=3)
            nc.tensor.matmul(ps_w[:, :], wtile[:, 0:128], wtile[:, 0:256],
                             start=True, stop=True)

        # ---------------- input loads (few, large DMAs) --------------------
        bqk_t = pool_a.tile([128, 4], f32, name="bqk_t", tag="bqk_t")
        nc.sync.dma_start(out=bqk_t[:, :], in_=bqk_e[:, :])
        # wqk packed k-block (cols 0:2048) then q-block; k loads first since
        # the first kpad needs only k
        wqk_t = pool_a.tile([128, KT * CQK], bf16, name="wqk_t", tag="wqk_t")
        nc.sync.dma_start(out=wqk_t[:, 0:2048], in_=wqk_e[:, 0:2048])
        xh = []
        for hb, xe in ((0, xh0_e), (1, xh1_e)):
            xt = pool_a.tile([128, KT * IH], bf16, name=f"xh{hb}", tag=f"xh{hb}")
            xh.append(xt)
        # prologue-critical loads only; the rest are issued after the
        # prologue so its rope-swap DMAs aren't queued behind bulk input
        cs_t = []
        csA_t = pool_a.tile([128, 2 * IH], bf16, name="csA", tag="csA")
        nc.sync.dma_start(out=xh[0][:, 0:4 * IH], in_=xh0_e[:, 0:4 * IH])
        nc.sync.dma_start(out=csA_t[:, :], in_=csA_e[:, :])
        nc.sync.dma_start(out=xh[0][:, 4 * IH:], in_=xh0_e[:, 4 * IH:])
        nc.sync.dma_start(out=wqk_t[:, 2048:], in_=wqk_e[:, 2048:])
        cs_t.append(csA_t)
        csB_t = pool_a.tile([128, 2 * IH], bf16, name="csB", tag="csB")
        cs_t.append(csB_t)
        wv_t = pool_a.tile([128, KT * CV], bf16, name="wv_t", tag="wv_t")
        wo_t = pool_live.tile([128, 2 * D], bf16, name="wo_t", tag="wo_t")

        # row 64 of all-ones: matmul lhsT/rhs must share a base partition,
        # and the denominator-reciprocal row lives on partition 64
        ones64 = pool_live.tile([65, 64], f32, name="ones64", tag="ones64")
        nc.vector.memset(ones64[64:65, :], 1.0)

        # ---------------- persistent SBUF tiles ----------------------------
        qk_rot = []
        for cb in range(4):
            qr = pool_live.tile([128, T], bf16, name=f"qkrot{cb}", tag=f"qkrot{cb}")
            qk_rot.append(qr)
        v_sb = []
        for tb in range(NJ):
            vt = pool_live.tile([128, HG, 65], bf16, name=f"v{tb}", tag=f"v{tb}")
            v_sb.append(vt)
        attn = []
        for pair in range(2):
            at = pool_live.tile([128, T], bf16, name=f"attn{pair}", tag=f"attn{pair}")
            attn.append(at)

        # ---------------- building blocks ----------------------------------
        def a_chunk_mm(cb, hb, dlo, dhi, ps_box):
            """q/k projection matmuls for d-chunks [dlo, dhi)."""
            if ps_box[0] is None:
                ps_box[0] = pool_ps.tile([128, IH], f32, name="ps2",
                                         tag="slot2", bufs=3)
            ps_qk = ps_box[0]
            for di in range(dlo, dhi):
                c0 = (di * 256 + (cb - 2) * 128) if cb >= 2 \
                    else (2048 + di * 256 + cb * 128)
                for half in range(2):
                    nc.tensor.matmul(
                        ps_qk[:, half * 512:(half + 1) * 512],
                        wqk_t[:, c0:c0 + 128],
                        xh[hb][:, di * IH + half * 512: di * IH + (half + 1) * 512],
                        start=(di == 0), stop=(di == KT - 1),
                    )

        def a_chunk_rope(cb, hb, ps_box):
            """RoPE + bias on the finished projection chunk."""
            ps_qk = ps_box[0]
            cst = cs_t[hb]
            bias = bqk_t[:, cb:cb + 1]
            # rot[p] = (q[p]+b[p])*cos[p] + swap((q+b)*sin_signed)[p]
            tmp1 = pool_a.tile([128, IH], f32, name="ropet1", tag="ropet1", bufs=2)
            tmps = pool_a.tile([128, IH], f32, name="tmps", tag="tmps", bufs=2)
            nc.vector.scalar_tensor_tensor(tmp1[:, :], ps_qk[:, :], bias,
                                           cst[:, 0:IH], op0=ADD, op1=MULT)
            nc.vector.scalar_tensor_tensor(tmps[:, :], ps_qk[:, :], bias,
                                           cst[:, IH:2 * IH], op0=ADD, op1=MULT)
            qsw = pool_a.tile([128, IH], f32, name="qsw", tag="qsw", bufs=2)
            for blk in range(2):
                b0 = blk * 64
                nc.sync.dma_start(out=qsw[b0:b0 + 32, :], in_=tmps[b0 + 32:b0 + 64, :])
                nc.sync.dma_start(out=qsw[b0 + 32:b0 + 64, :], in_=tmps[b0:b0 + 32, :])
            nc.vector.tensor_add(qk_rot[cb][:, hb * IH:(hb + 1) * IH],
                                 tmp1[:, :], qsw[:, :])
            ps_box[0] = None

        def a_chunk(cb, hb):
            box = [None]
            a_chunk_mm(cb, hb, 0, KT, box)
            a_chunk_rope(cb, hb, box)

        def a_parts(cb, hb):
            """a_chunk split into 3 filler closures."""
            box = [None]
            return [lambda: a_chunk_mm(cb, hb, 0, 4, box),
                    lambda: a_chunk_mm(cb, hb, 4, KT, box),
                    lambda: a_chunk_rope(cb, hb, box)]

        def b_unit(tb):
            """v projection for one t-block (no bias: folded to host)."""
            hb, c0 = tb // 8, (tb % 8) * 128
            ps_v = pool_ps.tile([128, CV], f32, name="psv", tag="slot2", bufs=3)
            for di in range(KT):
                nc.tensor.matmul(
                    ps_v[:, :],
                    xh[hb][:, di * IH + c0: di * IH + c0 + 128],
                    wv_t[:, di * CV:(di + 1) * CV],
                    start=(di == 0), stop=(di == KT - 1),
                )
            nc.vector.tensor_copy(
                v_sb[tb][:, :, 0:64],
                ps_v.rearrange("p (h d) -> p h d", h=HG),
            )
            nc.vector.memset(v_sb[tb][:, :, 64:65], 1.0)

        def d_unit(tb, copy_eng="v"):
            """out projection for one t-block; partial y to DRAM in bf16."""
            ps_y = pool_ps.tile([128, D], f32, name="psy", tag="slot2", bufs=3)
            for kb in range(2):
                for nb in range(D // 512):
                    nsl = slice(nb * 512, (nb + 1) * 512)
                    nc.tensor.matmul(
                        ps_y[:, nsl],
                        attn[kb][:, tb * 128:(tb + 1) * 128],
                        wo_t[:, kb * D + nb * 512: kb * D + (nb + 1) * 512],
                        start=(kb == 0), stop=(kb == 1),
                    )
            y_sb = pool_live.tile([128, D], bf16, name="y_sb", tag="y_sb", bufs=3)
            with nc.allow_low_precision("bf16 partial output; host sums in f32"):
                if copy_eng == "v":
                    nc.vector.tensor_copy(y_sb[:, :], ps_y[:, :])
                else:
                    nc.scalar.copy(y_sb[:, :], ps_y[:, :])
            nc.sync.dma_start(out=y_e[tb * 128:(tb + 1) * 128, :], in_=y_sb[:, :])

        kpads = {}

        def kpad_half(h, half):
            """Zero-padded k for head h, built one T-half at a time.

            One buffer per head (bufs=4), built once, reused by both ih
            units — rebuilds cost DVE time and stall the PE FIFO.
            """
            pair, hh = h // 2, h % 2
            hp = hh * 64
            if half == 0:
                kpads[h] = pool_live.tile([128, T], bf16, name="kpad",
                                          tag="kpad", bufs=4)
            kp = kpads[h]
            sl = slice(half * IH, (half + 1) * IH)
            op = 64 - hp
            nc.vector.memset(kp[op:op + 64, sl], 0.0)
            nc.vector.tensor_copy(kp[hp:hp + 64, sl],
                                  qk_rot[2 + pair][hp:hp + 64, sl])

        # stream state
        pt_tiles = {}
        ps_o_box = [None]

        def scores_exp(u, j):
            h, ih = u
            pair = h // 2
            ps_s = pool_ps.tile([128, IH], f32, name="pss", tag="slot2", bufs=3)
            pt = pool_live.tile([128, IH], bf16, name="pt", tag="pt",
                                bufs=MAXLAG + 4)
            kp = kpads[h]
            jsl = slice(j * 128, (j + 1) * 128)
            for nb in range(2):
                nsl = slice(nb * 512, (nb + 1) * 512)
                nc.tensor.matmul(ps_s[:, nsl], kp[:, jsl],
                                 qk_rot[pair][:, ih * IH + nb * 512:
                                              ih * IH + (nb + 1) * 512],
                                 start=True, stop=True)
            nc.scalar.activation(pt[:, :], ps_s[:, :], EXP, scale=0.125)
            pt_tiles[(u, j)] = pt

        def pv(u, j):
            h, ih = u
            if ps_o_box[0] is None:
                ps_o_box[0] = pool_ps.tile([65, IH], f32, name="pso",
                                           tag="ps_o", bufs=1)
            ps_o = ps_o_box[0]
            pt = pt_tiles.pop((u, j))
            for nb in range(2):
                nsl = slice(nb * 512, (nb + 1) * 512)
                nc.tensor.matmul(ps_o[:, nsl], v_sb[j][:, h, :], pt[:, nsl],
                                 start=(j == 0), stop=(j == NJ - 1))

        def normalize(u):
            h, ih = u
            pair, hh = h // 2, h % 2
            isl = slice(ih * IH, (ih + 1) * IH)
            ps_o = ps_o_box[0]
            # drain ps_o to SBUF first: frees the PV accumulator after one
            # copy (the next unit's PV j=0 reuses it almost immediately),
            # and gives the final multiply its one-PSUM-operand legality
            od = pool_live.tile([65, IH], f32, name="odrain", tag="odrain",
                                bufs=2)
            nc.vector.tensor_copy(od[:, :], ps_o[:, :])
            ps_o_box[0] = None
            rt = pool_live.tile([65, IH], f32, name="recip_t", tag="recip_t",
                                bufs=2)
            with nc.allow_low_precision("bf16 softmax normalization"):
                nc.vector.reciprocal_approx_fast(out=rt[:, :], in_=od[:, :])
            # broadcast the denominator-reciprocal row across 64 partitions
            # with a PE outer product (v1 used a DMA hop + gpsimd broadcast)
            ps_b = pool_ps.tile([64, IH], f32, name="psb", tag="slot2", bufs=3)
            for nb in range(2):
                nsl = slice(nb * 512, (nb + 1) * 512)
                nc.tensor.matmul(ps_b[:, nsl], ones64[64:65, :], rt[64:65, nsl],
                                 start=True, stop=True)
            with nc.allow_low_precision("bf16 attention output"):
                if hh == 0:
                    nc.vector.tensor_mul(attn[pair][0:64, isl],
                                         od[0:64, :], ps_b[:, :])
                else:
                    atmp = pool_live.tile([64, IH], bf16, name="atmp",
                                          tag="atmp", bufs=2)
                    nc.vector.tensor_mul(atmp[:, :], od[0:64, :], ps_b[:, :])
                    nc.sync.dma_start(out=attn[pair][64:128, isl], in_=atmp[:, :])

        # ---------------- prologue ------------------------------------------
        a_chunk(2, 0)        # k pair0, T-half 0
        kpad_half(0, 0)      # before a(0,0)'s rope in the DVE FIFO
        a_chunk(0, 0)        # q pair0, T-half 0
        # bulk loads not needed by the prologue, issued after its rope DMAs
        nc.sync.dma_start(out=xh[1][:, 0:4 * IH], in_=xh1_e[:, 0:4 * IH])
        nc.sync.dma_start(out=xh[1][:, 4 * IH:], in_=xh1_e[:, 4 * IH:])
        nc.sync.dma_start(out=wv_t[:, :], in_=wv_e[:, :])
        nc.sync.dma_start(out=csB_t[:, :], in_=csB_e[:, :])
        nc.sync.dma_start(out=wo_t[:, :], in_=wo_e[:, :])

        # ---------------- stream fillers, with PE-cost annotations ----------
        from collections import defaultdict
        fill = defaultdict(list)
        fcost = defaultdict(int)

        def put(p, c, cost=0):
            fill[p].append(c)
            fcost[p] += cost

        APART_NS = 1728   # 8 projection matmuls of N=512
        B_NS = 900        # b_unit: 8 matmuls of N=256 + copy
        D_NS = 950        # d_unit: 4 matmuls of N=512 + copy + dma

        pa = a_parts(2, 1)
        put(0, pa[0], APART_NS); put(2, pa[1], APART_NS); put(4, pa[2])
        put(3, lambda: kpad_half(1, 0))
        put(5, lambda: kpad_half(0, 1))
        put(6, lambda: kpad_half(1, 1))
        pa = a_parts(3, 0)
        put(7, pa[0], APART_NS); put(9, pa[1], APART_NS); put(11, pa[2])
        pa = a_parts(3, 1)
        put(12, pa[0], APART_NS); put(14, pa[1], APART_NS); put(16, pa[2])
        pa = a_parts(1, 0)
        put(18, pa[0], APART_NS); put(20, pa[1], APART_NS); put(22, pa[2])
        put(24, lambda: kpad_half(2, 0))
        put(26, lambda: kpad_half(2, 1))
        put(28, lambda: kpad_half(3, 0))
        put(30, lambda: kpad_half(3, 1))
        pa = a_parts(0, 1)
        put(50, pa[0], APART_NS); put(52, pa[1], APART_NS); put(54, pa[2])
        pa = a_parts(1, 1)
        put(68, pa[0], APART_NS); put(70, pa[1], APART_NS); put(72, pa[2])
        for tb in range(NJ):
            put(tb, lambda tb=tb: b_unit(tb), B_NS)
        # out-projection for t-blocks 0..5; idx 63's normalize is forced by
        # MAXLAG to land by p75, so p80+ can't head-of-line-block the PE
        for i, tb in enumerate(range(6)):
            put(80 + 5 * i, lambda tb=tb: d_unit(tb, "v"), D_NS)

        # ---------------- PV leveling plan ----------------------------------
        # defer PV matmuls out of overloaded early periods and catch up in
        # the slack, so per-period PE work tracks the ~1150ns exp cadence
        SCORES_NS = 432
        PV_NS = 432
        CAP = 1320
        MINLAG = 7   # below this, PV waits on its own exp and locksteps PE
        NP = 128
        pv_plan = defaultdict(list)
        ptr = 0
        p = 0
        while ptr < NP:
            pe = (SCORES_NS if p < NP else 0) + fcost[p]
            while ptr < NP and ptr + MINLAG <= p and len(pv_plan[p]) < 3:
                if (p - ptr) >= MAXLAG or pe + PV_NS <= CAP or p >= NP:
                    pv_plan[p].append(ptr)
                    pe += PV_NS
                    ptr += 1
                else:
                    break
            p += 1
        NPTOT = p

        # ---------------- the stream ----------------------------------------
        units = [(0, 0), (1, 0), (2, 0), (3, 0), (0, 1), (1, 1), (2, 1), (3, 1)]
        for p in range(NPTOT):
            if p < NP:
                scores_exp(units[p // NJ], p % NJ)
            for f in fill[p]:
                f()
            for idx in pv_plan[p]:
                uq = units[idx // NJ]
                pv(uq, idx % NJ)
                if idx % NJ == NJ - 1:
                    normalize(uq)

        # ---------------- tail ----------------------------------------------
        pool_a.release()
        for i, tb in enumerate(range(6, 16)):
            d_unit(tb, "v" if i % 2 == 0 else "s")

        pool_ps.release()
        pool_live.release()
        tc.__exit__(None, None, None)
        nc.finalize()
        return nc


def make_inputs(x, w_qkv, b_qkv, w_out):
    """Host-side shard prep. Returns in_maps list for the 8 cores."""
    half = HD // 2
    inv = 1.0 / (10000.0 ** (np.arange(half, dtype=np.float32) / half))
    fr = np.arange(T, dtype=np.float32)[:, None] * inv[None, :]   # [T, 32]
    cosT = np.cos(fr).T                                           # [32, T]
    sinT = np.sin(fr).T
    cos128 = np.tile(cosT, (4, 1)).astype(np.float32)             # [128, T]
    sin128 = np.tile(sinT, (4, 1)).astype(np.float32)
    sign = np.where((np.arange(128) % 64) < 32, 1.0, -1.0).astype(np.float32)
    sin128 = sin128 * sign[:, None]

    in_maps = []
    for c in range(N_CORES):
        b, g = c // 4, c % 4
        qcols = slice(g * 256, (g + 1) * 256)
        kcols = slice(D + g * 256, D + (g + 1) * 256)
        vcols = slice(2 * D + g * 256, 2 * D + (g + 1) * 256)

        wqk = np.concatenate([w_qkv[:, qcols], w_qkv[:, kcols]], axis=1)
        bqk = np.concatenate([b_qkv[qcols], b_qkv[kcols]])        # [512]
        wv = w_qkv[:, vcols]                                      # [D, 256]
        wo = w_out[g * 256:(g + 1) * 256, :]                      # [256, D]
        xT = np.ascontiguousarray(x[b].T).astype(BF16)            # [D, T]

        wqk_pack = np.empty((128, KT * CQK), dtype=BF16)
        wv_pack = np.empty((128, KT * CV), dtype=BF16)
        xh0 = np.empty((128, KT * IH), dtype=BF16)
        xh1 = np.empty((128, KT * IH), dtype=BF16)
        for di in range(KT):
            rs = slice(di * 128, (di + 1) * 128)
            # k-block (chunk cols 256:512) first, q-block second — the
            # device loads k first for the first kpad
            wqk_pack[:, di * 256:(di + 1) * 256] = wqk[rs, 256:512].astype(BF16)
            wqk_pack[:, 2048 + di * 256: 2048 + (di + 1) * 256] = \
                wqk[rs, 0:256].astype(BF16)
            wv_pack[:, di * CV:(di + 1) * CV] = wv[rs, :].astype(BF16)
            xh0[:, di * IH:(di + 1) * IH] = xT[rs, 0:IH]
            xh1[:, di * IH:(di + 1) * IH] = xT[rs, IH:T]
        bqkT = np.empty((128, 4), dtype=np.float32)
        for cb in range(4):
            bqkT[:, cb] = bqk[cb * 128:(cb + 1) * 128]
        csA = np.concatenate([cos128[:, 0:IH], sin128[:, 0:IH]],
                             axis=1).astype(BF16)
        csB = np.concatenate([cos128[:, IH:T], sin128[:, IH:T]],
                             axis=1).astype(BF16)
        wo_pack = np.empty((128, 2 * D), dtype=BF16)
        for kb in range(2):
            wo_pack[:, kb * D:(kb + 1) * D] = wo[kb * 128:(kb + 1) * 128, :]

        in_maps.append({
            "wqk": wqk_pack, "xh0": xh0, "xh1": xh1,
            "csA": csA, "csB": csB, "bqk": bqkT,
            "wv": wv_pack, "wo": wo_pack,
        })
    return in_maps


_NC_CACHE = [None]


def get_graph():
    if _NC_CACHE[0] is None:
        _NC_CACHE[0] = _Builder().build()
    return _NC_CACHE[0]


def kernel(x, w_qkv, b_qkv, w_out, b_out, _trace=False):
    from concourse.bass_utils import run_bass_kernel_spmd

    x = np.asarray(x)
    w_qkv = np.asarray(w_qkv)
    b_qkv = np.asarray(b_qkv)
    w_out = np.asarray(w_out)
    b_out = np.asarray(b_out)

    nc = get_graph()
    in_maps = make_inputs(x, w_qkv, b_qkv, w_out)
    kw = {}
    if _trace:
        _install_ntff_shim()
        kw = {"trace": True}
    res = run_bass_kernel_spmd(nc, in_maps, core_ids=list(range(N_CORES)), **kw)

    # v bias folded here: softmax rows sum to 1, so bias_v contributes
    # exactly bv^T @ w_out to every token's output.
    bias_row = (b_qkv[2 * D:].astype(np.float64) @
                w_out.astype(np.float64)).astype(np.float32) + b_out
    out = np.empty((B, T, D), dtype=np.float32)
    for b in range(B):
        acc = np.asarray(res.results[4 * b]["y"]).astype(np.float32)
        for g in range(1, 4):
            acc += np.asarray(res.results[4 * b + g]["y"]).astype(np.float32)
        out[b] = acc + bias_row[None, :]
    if _trace:
        kernel.last_exec_time_ns = res.exec_time_ns
        kernel.last_result = res
    return out


def _install_ntff_shim():
    """The agent image's antenv lacks axon_hooks; shim it so trace=True works."""
    import types
    if "antenv.axon_hooks" in sys.modules:
        return
    try:
        from trn_agent_boot.trn_boot import _ntff_profile_via_ctypes
        hook = _ntff_profile_via_ctypes("/opt/axon/libaxon_pjrt.so")
    except Exception:
        hook = None
    mod = types.ModuleType("antenv.axon_hooks")
    _h = [hook]
    mod.set_axon_ntff_profile_hook = lambda h: _h.__setitem__(0, h)
    mod.get_axon_ntff_profile_hook = lambda: _h[0]
    sys.modules["antenv.axon_hooks"] = mod
